# revision 3
# baseline (speedup 1.0000x reference)
"""Trainium2 Bass kernel for nn_CrossAttentionFusion (B=4, S=2048, D=512, H=8).

Sharding: 8 cores = 4 batches x 2 head-groups (4 heads each). Each core
receives its batch's value/key/query [2048, 512] (bf16) plus its head-group's
weight rows W*[256, 512] (bf16) / biases [256] (f32), and produces out [256]
(the 4 heads' head_logits). Host folds the 1/sqrt(64) softmax scale into
Wq/bq and multiplies the gathered output by 8 to compensate.

Per-core math, per head h (d = head dim 64, i/j = sequence):
  qT/kT/vT [64, 2048] = W_h @ inputT (+bias)        (T layouts throughout)
  sk[i,j] = sum_d qT[d,i] kT[d,j]   (already scaled) ; sv likewise with vT
  P~ = exp(s), Z[i] = sum_j P~[i,j]  (ACT fused accumulate, no max-sub:
       scores are ~N(0,1) so exp is safely bounded)
  GT[d,j] = sum_i (q_nat[i,d]/Zk[i]) P~k[i,j] + (q_nat[i,d]/Zv[i]) P~v[i,j]
  out[d]  = sum_j (kT+vT)[d,j] * GT[d,j]            (one tensor_tensor_reduce)

Structure (v2): inputs ship as bf16 and land pre-transposed via XBAR
dma_start_transpose (no PE transposes, no drain copies). The attention loop
is ACT-bound (exp); the two [128,1024] PSUM score buffers ping-pong at
(h, jh-half) granularity so exp runs back-to-back, and the GT matmuls are
software-pipelined one iteration behind so they never gate the next exp.
"""

import os
import sys

import numpy as np

if "/opt/trn_rl_repo" not in sys.path and os.path.isdir("/opt/trn_rl_repo"):
    sys.path.insert(0, "/opt/trn_rl_repo")

from contextlib import ExitStack

import concourse.bass as bass
import concourse.mybir as mybir
import concourse.tile as tile
from concourse import bacc

B, S, D, H, HD = 4, 2048, 512, 8, 64
DG = 256  # local output dims per core (4 heads x 64)
f32 = mybir.dt.float32
f32r = mybir.dt.float32r
bf16 = mybir.dt.bfloat16
FT = mybir.ActivationFunctionType
ALU = mybir.AluOpType
AXL = mybir.AxisListType


def build_program(phase=4, reps=1, timing=False):
    # phase: 1=projections only, 2=+scores/exp, >=3=full
    # reps: emit the whole computation N times (timing: marginal cost/rep)
    # timing: kept for compatibility; the program is identical
    nc = bacc.Bacc("TRN2", target_bir_lowering=False)

    val = nc.dram_tensor("value", [S, D], bf16, kind="ExternalInput")
    key = nc.dram_tensor("key", [S, D], bf16, kind="ExternalInput")
    qry = nc.dram_tensor("query", [S, D], bf16, kind="ExternalInput")
    Wv = nc.dram_tensor("Wv", [DG, D], bf16, kind="ExternalInput")
    bv = nc.dram_tensor("bv", [DG], f32, kind="ExternalInput")
    Wk = nc.dram_tensor("Wk", [DG, D], bf16, kind="ExternalInput")
    bk = nc.dram_tensor("bk", [DG], f32, kind="ExternalInput")
    Wq = nc.dram_tensor("Wq", [DG, D], bf16, kind="ExternalInput")
    bq = nc.dram_tensor("bq", [DG], f32, kind="ExternalInput")
    out = nc.dram_tensor("out", [DG], f32, kind="ExternalOutput")

    with tile.TileContext(nc) as tc, ExitStack() as ctx:
        for rep in range(reps):
          if rep > 0:
              tc.strict_bb_all_engine_barrier()
          with ExitStack() as rctx:
            qkv = rctx.enter_context(tc.tile_pool(name=f"qkv_{rep}", bufs=1))
            qT2 = [qkv.tile([128, S], f32r, name=f"qT2_{p}_{rep}") for p in (0, 1)]
            kT2 = [qkv.tile([128, S], f32r, name=f"kT2_{p}_{rep}") for p in (0, 1)]
            vT2 = [qkv.tile([128, S], f32r, name=f"vT2_{p}_{rep}") for p in (0, 1)]
            fus = [qkv.tile([128, S], f32, name=f"fus_{p}_{rep}") for p in (0, 1)]
            qn = [qkv.tile([128, 16, 128], bf16, name=f"qn_{p}_{rep}") for p in (0, 1)]
            outsb = qkv.tile([128, 2], f32, name=f"outsb_{rep}")

            # --- weights + inputs: XBAR-transposing DMAs straight from DRAM ---
            wT = {}
            bias = {}
            for nm, wdram, bdram in (("v", Wv, bv), ("k", Wk, bk), ("q", Wq, bq)):
                bt = qkv.tile([128, 2], f32, name=f"b{nm}_{rep}")
                nc.sync.dma_start(bt, bdram[:].rearrange("(t p) -> p t", p=128))
                bias[nm] = bt
                wt = qkv.tile([128, 4, DG], bf16, name=f"wT{nm}_{rep}")
                nc.sync.dma_start_transpose(wt, wdram[:, :])
                wT[nm] = wt
            inT = {}
            for nm, dram in (("k", key), ("q", qry), ("v", val)):
                it = qkv.tile([128, 4, S], bf16, name=f"inT{nm}_{rep}")
                nc.sync.dma_start_transpose(it, dram[:, :])
                inT[nm] = it

            # --- projections: dstT2[p][c, s] = sum_cin W[c, cin] inT[cin, s] ---
            def proj(nm, dstT2):
                with tc.tile_pool(
                    name=f"ps_{nm}_{rep}", bufs=4, space="PSUM"
                ) as tps:
                    for p in (0, 1):
                        for jb in range(4):
                            ps = tps.tile([128, 512], f32, tag="pj")
                            for cc in range(4):
                                nc.tensor.matmul(
                                    ps,
                                    wT[nm][:, cc, p * 128 : (p + 1) * 128],
                                    inT[nm][:, cc, jb * 512 : (jb + 1) * 512],
                                    start=(cc == 0),
                                    stop=(cc == 3),
                                )
                            nc.vector.tensor_scalar_add(
                                dstT2[p][:, jb * 512 : (jb + 1) * 512],
                                ps,
                                bias[nm][:, p : p + 1],
                            )

            proj("k", kT2)
            proj("q", qT2)
            # natural-layout q (bf16) for the 1/Z row scaling: convert + XBAR
            with tc.tile_pool(name=f"qb_{rep}", bufs=2) as qbp:
                for p in (0, 1):
                    qb = qbp.tile([128, S], bf16, tag="qb")
                    nc.vector.tensor_copy(qb, qT2[p].bitcast(f32))
                    nc.sync.dma_start_transpose(qn[p], qb)
            proj("v", vT2)
            for p in (0, 1):
                nc.vector.tensor_add(fus[p], kT2[p].bitcast(f32), vT2[p].bitcast(f32))

            # --- attention: m-outer, sc ping-pong, GT pipelined one iter back ---
            for p in (0, 1):
                with (
                    tc.tile_pool(name=f"aps{p}_{rep}", bufs=1, space="PSUM") as aps,
                    tc.tile_pool(name=f"pp{p}_{rep}", bufs=6) as ppool,
                    tc.tile_pool(name=f"sm{p}_{rep}", bufs=3) as smp,
                ):
                    gt2 = aps.tile([128, S], f32, name=f"gt{p}_{rep}")
                    sc = [
                        aps.tile([128, 1024], f32, name=f"sc{p}{h}_{rep}")
                        for h in (0, 1)
                    ]
                    prev = None  # (wsd0, wsd1, pts0, pts1, first)

                    def emit_gt(pv, last):
                        w0, w1, pt0, pt1, first = pv
                        for jq in range(4):
                            for h, wh, pth in ((0, w0, pt0), (1, w1, pt1)):
                                nc.tensor.matmul(
                                    gt2[64 * h : 64 * (h + 1), jq * 512 : (jq + 1) * 512],
                                    wh,
                                    pth[:, jq * 512 : (jq + 1) * 512],
                                    start=first,
                                    stop=last,
                                    skip_group_check=True,
                                )

                    if phase >= 2:
                        for m, src in ((0, kT2[p]), (1, vT2[p])):
                            for ic in range(16):
                                pts = [
                                    ppool.tile(
                                        [128, S], bf16, tag=f"pt{h}",
                                        name=f"pt{h}_{m}_{ic}_{rep}",
                                    )
                                    for h in (0, 1)
                                ]
                                zp = smp.tile(
                                    [128, 4], f32, tag="zp", name=f"zp_{m}_{ic}_{rep}"
                                )
                                lhs = [
                                    qT2[p][64 * h : 64 * (h + 1), ic * 128 : (ic + 1) * 128]
                                    for h in (0, 1)
                                ]

                                def mmpair(h, jh):
                                    for jq in (0, 1):
                                        nc.tensor.matmul(
                                            sc[h][:, jq * 512 : (jq + 1) * 512],
                                            lhs[h],
                                            src[
                                                64 * h : 64 * (h + 1),
                                                (2 * jh + jq) * 512 : (2 * jh + jq + 1) * 512,
                                            ],
                                            start=True,
                                            stop=True,
                                        )

                                def expi(h, jh):
                                    nc.scalar.activation(
                                        pts[h][:, jh * 1024 : (jh + 1) * 1024],
                                        sc[h],
                                        FT.Exp,
                                        accum_out=zp[:, 2 * h + jh : 2 * h + jh + 1],
                                    )

                                mmpair(0, 0)
                                mmpair(1, 0)
                                expi(0, 0)
                                if prev is not None and phase >= 3:
                                    emit_gt(prev, last=False)
                                mmpair(0, 1)
                                expi(1, 0)
                                mmpair(1, 1)
                                expi(0, 1)
                                expi(1, 1)
                                # Z per head: zp cols (2h+jh) -> zs2[:, h]
                                zp3 = zp.rearrange("p (a b) -> p a b", b=2)
                                zs2 = smp.tile(
                                    [128, 2], f32, tag="zs", name=f"zs_{m}_{ic}_{rep}"
                                )
                                nc.vector.tensor_tensor(
                                    zs2, zp3[:, :, 0], zp3[:, :, 1], ALU.add
                                )
                                rs2 = smp.tile(
                                    [128, 2], f32, tag="rs", name=f"rs_{m}_{ic}_{rep}"
                                )
                                nc.vector.reciprocal(rs2, zs2)
                                wsd = []
                                for h in (0, 1):
                                    w = smp.tile(
                                        [128, HD], bf16, tag=f"w{h}",
                                        name=f"w{h}_{m}_{ic}_{rep}",
                                    )
                                    nc.vector.tensor_scalar_mul(
                                        w,
                                        qn[p][:, ic, 64 * h : 64 * (h + 1)],
                                        rs2[:, h : h + 1],
                                    )
                                    wsd.append(w)
                                prev = (
                                    wsd[0], wsd[1], pts[0], pts[1],
                                    (m == 0 and ic == 0),
                                )
                        if phase >= 3:
                            emit_gt(prev, last=True)
                            prev = None

                    scr = smp.tile([128, S], f32, tag="scr")
                    nc.vector.tensor_mul(
                        scr, gt2 if phase >= 3 else fus[p], fus[p]
                    )
                    nc.vector.tensor_reduce(
                        outsb[:, p : p + 1], scr, axis=AXL.X, op=ALU.add
                    )
                nc.sync.dma_start(
                    out[:].rearrange("(t q) -> t q", t=2)[p].unsqueeze(1),
                    outsb[:, p : p + 1],
                )

    nc.compile()
    return nc


_CACHE = {}


def _program(phase=4, reps=1, timing=False):
    key = (phase, reps, timing)
    if key not in _CACHE:
        _CACHE[key] = build_program(phase=phase, reps=reps, timing=timing)
    return _CACHE[key]


def make_in_maps(inputs):
    import ml_dtypes

    b16 = ml_dtypes.bfloat16
    v = np.asarray(inputs["value"], dtype=np.float32).astype(b16)
    k = np.asarray(inputs["key"], dtype=np.float32).astype(b16)
    q = np.asarray(inputs["query"], dtype=np.float32).astype(b16)
    Wv = np.asarray(inputs["Wv"], dtype=np.float32)
    Wk = np.asarray(inputs["Wk"], dtype=np.float32)
    Wq = np.asarray(inputs["Wq"], dtype=np.float32)
    bv = np.asarray(inputs["bv"], dtype=np.float32)
    bk = np.asarray(inputs["bk"], dtype=np.float32)
    bq = np.asarray(inputs["bq"], dtype=np.float32)
    in_maps = []
    for c in range(8):
        b, g = divmod(c, 2)
        sl = slice(g * DG, (g + 1) * DG)
        in_maps.append(
            {
                "value": np.ascontiguousarray(v[b]),
                "key": np.ascontiguousarray(k[b]),
                "query": np.ascontiguousarray(q[b]),
                "Wv": np.ascontiguousarray(Wv[sl]).astype(b16),
                "bv": np.ascontiguousarray(bv[sl]),
                "Wk": np.ascontiguousarray(Wk[sl]).astype(b16),
                "bk": np.ascontiguousarray(bk[sl]),
                # softmax 1/sqrt(HD) folded into the query projection
                "Wq": (np.ascontiguousarray(Wq[sl]) * 0.125).astype(b16),
                "bq": np.ascontiguousarray(bq[sl]) * 0.125,
            }
        )
    return in_maps


def gather_out(results):
    out = np.zeros((B, H * HD), np.float32)
    for c in range(8):
        b, g = divmod(c, 2)
        # compensate the folded 1/8 query scale
        out[b, g * DG : (g + 1) * DG] = results[c]["out"] * 8.0
    return out


def run_sharded(inputs, trace=False, **kwargs):
    from concourse.bass_utils import run_bass_kernel_spmd

    nc = _program()
    res = run_bass_kernel_spmd(
        nc, make_in_maps(inputs), core_ids=list(range(8)), trace=trace, **kwargs
    )
    return gather_out(res.results), res


def kernel(**inputs):
    out, _ = run_sharded(inputs)
    return out


# revision 7
# speedup vs baseline: 1.5203x; 1.5203x over previous
"""Trainium2 Bass kernel for nn_CrossAttentionFusion (B=4, S=2048, D=512, H=8).

Sharding: 8 cores = 4 batches x 2 head-groups (4 heads each). Each core
receives its batch's value/key/query [2048, 512] (bf16) plus its head-group's
weight rows W*[256, 512] (bf16) / biases [256] (f32), and produces out [256]
(the 4 heads' head_logits). Host folds the 1/sqrt(64) softmax scale into
Wq/bq and multiplies the gathered output by 8 to compensate.

Per-core math, per head h (d = head dim 64, i/j = sequence):
  qT/kT/vT [64, 2048] = W_h @ inputT (+bias)        (T layouts throughout)
  sk[i,j] = sum_d qT[d,i] kT[d,j]   (already scaled) ; sv likewise with vT
  P~ = exp(s), Z[i] = sum_j P~[i,j]  (ACT fused accumulate, no max-sub:
       scores are ~N(0,1) so exp is safely bounded)
  GT[d,j] = sum_i (q_nat[i,d]/Zk[i]) P~k[i,j] + (q_nat[i,d]/Zv[i]) P~v[i,j]
  out[d]  = sum_j (kT+vT)[d,j] * GT[d,j]            (one tensor_tensor_reduce)

Structure (v2): inputs ship as bf16 and land pre-transposed via XBAR
dma_start_transpose (no PE transposes, no drain copies). The attention loop
is ACT-bound (exp); the two [128,1024] PSUM score buffers ping-pong at
(h, jh-half) granularity so exp runs back-to-back, and the GT matmuls are
software-pipelined one iteration behind so they never gate the next exp.
"""

import os
import sys

import numpy as np

if "/opt/trn_rl_repo" not in sys.path and os.path.isdir("/opt/trn_rl_repo"):
    sys.path.insert(0, "/opt/trn_rl_repo")

from contextlib import ExitStack

import concourse.bass as bass
import concourse.mybir as mybir
import concourse.tile as tile
from concourse import bacc

B, S, D, H, HD = 4, 2048, 512, 8, 64
DG = 256  # local output dims per core (4 heads x 64)
f32 = mybir.dt.float32
f32r = mybir.dt.float32r
bf16 = mybir.dt.bfloat16
FT = mybir.ActivationFunctionType
ALU = mybir.AluOpType
AXL = mybir.AxisListType


def build_program(phase=4, reps=1, timing=False):
    # phase: 1=projections only, 2=+scores/exp, >=3=full
    # reps: emit the whole computation N times (timing: marginal cost/rep)
    # timing: kept for compatibility; the program is identical
    nc = bacc.Bacc("TRN2", target_bir_lowering=False)

    val = nc.dram_tensor("value", [S, D], bf16, kind="ExternalInput")
    key = nc.dram_tensor("key", [S, D], bf16, kind="ExternalInput")
    qry = nc.dram_tensor("query", [S, D], bf16, kind="ExternalInput")
    Wv = nc.dram_tensor("Wv", [DG, D], bf16, kind="ExternalInput")
    bv = nc.dram_tensor("bv", [DG], f32, kind="ExternalInput")
    Wk = nc.dram_tensor("Wk", [DG, D], bf16, kind="ExternalInput")
    bk = nc.dram_tensor("bk", [DG], f32, kind="ExternalInput")
    Wq = nc.dram_tensor("Wq", [DG, D], bf16, kind="ExternalInput")
    bq = nc.dram_tensor("bq", [DG], f32, kind="ExternalInput")
    out = nc.dram_tensor("out", [DG], f32, kind="ExternalOutput")

    with tile.TileContext(nc) as tc, ExitStack() as ctx:
        for rep in range(reps):
          if rep > 0:
              tc.strict_bb_all_engine_barrier()
          with ExitStack() as rctx:
            qkv = rctx.enter_context(tc.tile_pool(name=f"qkv_{rep}", bufs=1))
            qT2 = [qkv.tile([128, S], f32r, name=f"qT2_{p}_{rep}") for p in (0, 1)]
            kT2 = [qkv.tile([128, S], f32r, name=f"kT2_{p}_{rep}") for p in (0, 1)]
            vT2 = [qkv.tile([128, S], f32r, name=f"vT2_{p}_{rep}") for p in (0, 1)]
            fus = [qkv.tile([128, S], f32, name=f"fus_{p}_{rep}") for p in (0, 1)]
            qn = [qkv.tile([128, 16, 128], bf16, name=f"qn_{p}_{rep}") for p in (0, 1)]
            outsb = qkv.tile([128, 2], f32, name=f"outsb_{rep}")

            # --- weights + inputs: XBAR-transposing DMAs straight from DRAM ---
            wT = {}
            bias = {}
            for nm, wdram, bdram in (("v", Wv, bv), ("k", Wk, bk), ("q", Wq, bq)):
                bt = qkv.tile([128, 2], f32, name=f"b{nm}_{rep}")
                nc.sync.dma_start(bt, bdram[:].rearrange("(t p) -> p t", p=128))
                bias[nm] = bt
                wt = qkv.tile([128, 4, DG], bf16, name=f"wT{nm}_{rep}")
                nc.sync.dma_start_transpose(wt, wdram[:, :])
                wT[nm] = wt
            inT = {}
            for nm, dram in (("k", key), ("q", qry), ("v", val)):
                it = qkv.tile([128, 4, S], bf16, name=f"inT{nm}_{rep}")
                nc.sync.dma_start_transpose(it, dram[:, :])
                inT[nm] = it

            # --- projections: dstT2[p][c, s] = sum_cin W[c, cin] inT[cin, s] ---
            def proj(nm, dstT2):
                with tc.tile_pool(
                    name=f"ps_{nm}_{rep}", bufs=4, space="PSUM"
                ) as tps:
                    for p in (0, 1):
                        for jb in range(4):
                            ps = tps.tile([128, 512], f32, tag="pj")
                            for cc in range(4):
                                nc.tensor.matmul(
                                    ps,
                                    wT[nm][:, cc, p * 128 : (p + 1) * 128],
                                    inT[nm][:, cc, jb * 512 : (jb + 1) * 512],
                                    start=(cc == 0),
                                    stop=(cc == 3),
                                )
                            nc.vector.tensor_scalar_add(
                                dstT2[p][:, jb * 512 : (jb + 1) * 512],
                                ps,
                                bias[nm][:, p : p + 1],
                            )

            proj("k", kT2)
            proj("q", qT2)
            # natural-layout q (bf16) for the 1/Z row scaling: convert + XBAR
            with tc.tile_pool(name=f"qb_{rep}", bufs=2) as qbp:
                for p in (0, 1):
                    qb = qbp.tile([128, S], bf16, tag="qb")
                    nc.vector.tensor_copy(qb, qT2[p].bitcast(f32))
                    nc.sync.dma_start_transpose(qn[p], qb)
            proj("v", vT2)
            for p in (0, 1):
                nc.vector.tensor_add(fus[p], kT2[p].bitcast(f32), vT2[p].bitcast(f32))

            # --- attention: m-outer, sc ping-pong, GT pipelined one iter back ---
            for p in (0, 1):
                with (
                    tc.tile_pool(name=f"aps{p}_{rep}", bufs=1, space="PSUM") as aps,
                    tc.tile_pool(name=f"pp{p}_{rep}", bufs=6) as ppool,
                    tc.tile_pool(name=f"sm{p}_{rep}", bufs=3) as smp,
                ):
                    gt2 = aps.tile([128, S], f32, name=f"gt{p}_{rep}")
                    sc = [
                        aps.tile([128, 1024], f32, name=f"sc{p}{h}_{rep}")
                        for h in (0, 1)
                    ]
                    prev = None  # (wsd0, wsd1, pts0, pts1, first)

                    def emit_gt(pv, last):
                        w0, w1, pt0, pt1, first = pv
                        for jq in range(4):
                            for h, wh, pth in ((0, w0, pt0), (1, w1, pt1)):
                                nc.tensor.matmul(
                                    gt2[64 * h : 64 * (h + 1), jq * 512 : (jq + 1) * 512],
                                    wh,
                                    pth[:, jq * 512 : (jq + 1) * 512],
                                    start=first,
                                    stop=last,
                                    skip_group_check=True,
                                )

                    # probe phases: 12=score mms only; 13=+exp, no accum/Z/wsd;
                    # 14=+exp with accum, no wsd/GT; 2=no GT; >=3(or 4)=full
                    if phase >= 2:
                        for m, src in ((0, kT2[p]), (1, vT2[p])):
                            for ic in range(16):
                                pts = [
                                    ppool.tile(
                                        [128, S], bf16, tag=f"pt{h}",
                                        name=f"pt{h}_{m}_{ic}_{rep}",
                                    )
                                    for h in (0, 1)
                                ]
                                zp = smp.tile(
                                    [128, 4], f32, tag="zp", name=f"zp_{m}_{ic}_{rep}"
                                )
                                lhs = [
                                    qT2[p][64 * h : 64 * (h + 1), ic * 128 : (ic + 1) * 128]
                                    for h in (0, 1)
                                ]

                                def mmpair(h, jh):
                                    for jq in (0, 1):
                                        nc.tensor.matmul(
                                            sc[h][:, jq * 512 : (jq + 1) * 512],
                                            lhs[h],
                                            src[
                                                64 * h : 64 * (h + 1),
                                                (2 * jh + jq) * 512 : (2 * jh + jq + 1) * 512,
                                            ],
                                            start=True,
                                            stop=True,
                                        )

                                def expi(h, jh):
                                    if phase == 12:
                                        return
                                    acc = None
                                    if phase != 13:
                                        acc = zp[:, 2 * h + jh : 2 * h + jh + 1]
                                    nc.scalar.activation(
                                        pts[h][:, jh * 1024 : (jh + 1) * 1024],
                                        sc[h],
                                        FT.Exp,
                                        accum_out=acc,
                                    )

                                mmpair(0, 0)
                                mmpair(1, 0)
                                expi(0, 0)
                                if prev is not None and phase >= 3 and phase < 10:
                                    emit_gt(prev, last=False)
                                mmpair(0, 1)
                                expi(1, 0)
                                mmpair(1, 1)
                                expi(0, 1)
                                expi(1, 1)
                                if phase in (12, 13, 14):
                                    continue
                                # Z per head: zp cols (2h+jh) -> zs2[:, h]
                                zp3 = zp.rearrange("p (a b) -> p a b", b=2)
                                zs2 = smp.tile(
                                    [128, 2], f32, tag="zs", name=f"zs_{m}_{ic}_{rep}"
                                )
                                nc.vector.tensor_tensor(
                                    zs2, zp3[:, :, 0], zp3[:, :, 1], ALU.add
                                )
                                rs2 = smp.tile(
                                    [128, 2], f32, tag="rs", name=f"rs_{m}_{ic}_{rep}"
                                )
                                nc.vector.reciprocal(rs2, zs2)
                                wsd = []
                                for h in (0, 1):
                                    w = smp.tile(
                                        [128, HD], bf16, tag=f"w{h}",
                                        name=f"w{h}_{m}_{ic}_{rep}",
                                    )
                                    nc.vector.tensor_scalar_mul(
                                        w,
                                        qn[p][:, ic, 64 * h : 64 * (h + 1)],
                                        rs2[:, h : h + 1],
                                    )
                                    wsd.append(w)
                                prev = (
                                    wsd[0], wsd[1], pts[0], pts[1],
                                    (m == 0 and ic == 0),
                                )
                        if phase >= 3 and phase < 10:
                            emit_gt(prev, last=True)
                        prev = None

                    scr = smp.tile([128, S], f32, tag="scr")
                    nc.vector.tensor_mul(
                        scr, gt2 if (3 <= phase < 10) else fus[p], fus[p]
                    )
                    nc.vector.tensor_reduce(
                        outsb[:, p : p + 1], scr, axis=AXL.X, op=ALU.add
                    )
                nc.sync.dma_start(
                    out[:].rearrange("(t q) -> t q", t=2)[p].unsqueeze(1),
                    outsb[:, p : p + 1],
                )

    nc.compile()
    return nc


_CACHE = {}


def _program(phase=4, reps=1, timing=False):
    key = (phase, reps, timing)
    if key not in _CACHE:
        _CACHE[key] = build_program(phase=phase, reps=reps, timing=timing)
    return _CACHE[key]


def make_in_maps(inputs):
    import ml_dtypes

    b16 = ml_dtypes.bfloat16
    v = np.asarray(inputs["value"], dtype=np.float32).astype(b16)
    k = np.asarray(inputs["key"], dtype=np.float32).astype(b16)
    q = np.asarray(inputs["query"], dtype=np.float32).astype(b16)
    Wv = np.asarray(inputs["Wv"], dtype=np.float32)
    Wk = np.asarray(inputs["Wk"], dtype=np.float32)
    Wq = np.asarray(inputs["Wq"], dtype=np.float32)
    bv = np.asarray(inputs["bv"], dtype=np.float32)
    bk = np.asarray(inputs["bk"], dtype=np.float32)
    bq = np.asarray(inputs["bq"], dtype=np.float32)
    in_maps = []
    for c in range(8):
        b, g = divmod(c, 2)
        sl = slice(g * DG, (g + 1) * DG)
        in_maps.append(
            {
                "value": np.ascontiguousarray(v[b]),
                "key": np.ascontiguousarray(k[b]),
                "query": np.ascontiguousarray(q[b]),
                "Wv": np.ascontiguousarray(Wv[sl]).astype(b16),
                "bv": np.ascontiguousarray(bv[sl]),
                "Wk": np.ascontiguousarray(Wk[sl]).astype(b16),
                "bk": np.ascontiguousarray(bk[sl]),
                # softmax 1/sqrt(HD) folded into the query projection
                "Wq": (np.ascontiguousarray(Wq[sl]) * 0.125).astype(b16),
                "bq": np.ascontiguousarray(bq[sl]) * 0.125,
            }
        )
    return in_maps


def gather_out(results):
    out = np.zeros((B, H * HD), np.float32)
    for c in range(8):
        b, g = divmod(c, 2)
        # compensate the folded 1/8 query scale
        out[b, g * DG : (g + 1) * DG] = results[c]["out"] * 8.0
    return out


def run_sharded(inputs, trace=False, **kwargs):
    from concourse.bass_utils import run_bass_kernel_spmd

    nc = _program()
    res = run_bass_kernel_spmd(
        nc, make_in_maps(inputs), core_ids=list(range(8)), trace=trace, **kwargs
    )
    return gather_out(res.results), res


def kernel(**inputs):
    out, _ = run_sharded(inputs)
    return out


# revision 8
# speedup vs baseline: 1.5274x; 1.0047x over previous
"""Trainium2 Bass kernel for nn_CrossAttentionFusion (B=4, S=2048, D=512, H=8).

Sharding: 8 cores = 4 batches x 2 head-groups (4 heads each). Each core
receives its batch's value/key/query [2048, 512] (bf16) plus its head-group's
weight rows W*[256, 512] (bf16) / biases [256] (f32), and produces out [256]
(the 4 heads' head_logits). Host folds the 1/sqrt(64) softmax scale into
Wq/bq and multiplies the gathered output by 8 to compensate.

Per-core math, per head h (d = head dim 64, i/j = sequence):
  qT/kT/vT [64, 2048] = W_h @ inputT (+bias)        (T layouts throughout)
  sk[i,j] = sum_d qT[d,i] kT[d,j]   (already scaled) ; sv likewise with vT
  P~ = exp(s), Z[i] = sum_j P~[i,j]  (ACT fused accumulate, no max-sub:
       scores are ~N(0,1) so exp is safely bounded)
  GT[d,j] = sum_i (q_nat[i,d]/Zk[i]) P~k[i,j] + (q_nat[i,d]/Zv[i]) P~v[i,j]
  out[d]  = sum_j (kT+vT)[d,j] * GT[d,j]            (one tensor_tensor_reduce)

Structure (v2): inputs ship as bf16 and land pre-transposed via XBAR
dma_start_transpose (no PE transposes, no drain copies). The attention loop
is ACT-bound (exp); the two [128,1024] PSUM score buffers ping-pong at
(h, jh-half) granularity so exp runs back-to-back, and the GT matmuls are
software-pipelined one iteration behind so they never gate the next exp.
"""

import os
import sys

import numpy as np

if "/opt/trn_rl_repo" not in sys.path and os.path.isdir("/opt/trn_rl_repo"):
    sys.path.insert(0, "/opt/trn_rl_repo")

from contextlib import ExitStack

import concourse.bass as bass
import concourse.mybir as mybir
import concourse.tile as tile
from concourse import bacc

B, S, D, H, HD = 4, 2048, 512, 8, 64
DG = 256  # local output dims per core (4 heads x 64)
f32 = mybir.dt.float32
f32r = mybir.dt.float32r
bf16 = mybir.dt.bfloat16
FT = mybir.ActivationFunctionType
ALU = mybir.AluOpType
AXL = mybir.AxisListType


def build_program(phase=4, reps=1, timing=False):
    # phase: 1=projections only, 2=+scores/exp, >=3=full
    # reps: emit the whole computation N times (timing: marginal cost/rep)
    # timing: kept for compatibility; the program is identical
    nc = bacc.Bacc("TRN2", target_bir_lowering=False)

    val = nc.dram_tensor("value", [S, D], bf16, kind="ExternalInput")
    key = nc.dram_tensor("key", [S, D], bf16, kind="ExternalInput")
    qry = nc.dram_tensor("query", [S, D], bf16, kind="ExternalInput")
    Wv = nc.dram_tensor("Wv", [DG, D], bf16, kind="ExternalInput")
    bv = nc.dram_tensor("bv", [DG], f32, kind="ExternalInput")
    Wk = nc.dram_tensor("Wk", [DG, D], bf16, kind="ExternalInput")
    bk = nc.dram_tensor("bk", [DG], f32, kind="ExternalInput")
    Wq = nc.dram_tensor("Wq", [DG, D], bf16, kind="ExternalInput")
    bq = nc.dram_tensor("bq", [DG], f32, kind="ExternalInput")
    out = nc.dram_tensor("out", [DG], f32, kind="ExternalOutput")

    with tile.TileContext(nc) as tc, ExitStack() as ctx:
        for rep in range(reps):
          if rep > 0:
              tc.strict_bb_all_engine_barrier()
          with ExitStack() as rctx:
            qkv = rctx.enter_context(tc.tile_pool(name=f"qkv_{rep}", bufs=1))
            qT2 = [qkv.tile([128, S], f32r, name=f"qT2_{p}_{rep}") for p in (0, 1)]
            kT2 = [qkv.tile([128, S], f32r, name=f"kT2_{p}_{rep}") for p in (0, 1)]
            vT2 = [qkv.tile([128, S], f32r, name=f"vT2_{p}_{rep}") for p in (0, 1)]
            fus = [qkv.tile([128, S], f32, name=f"fus_{p}_{rep}") for p in (0, 1)]
            qn = [qkv.tile([128, 16, 128], bf16, name=f"qn_{p}_{rep}") for p in (0, 1)]
            outsb = qkv.tile([128, 2], f32, name=f"outsb_{rep}")

            # --- weights + inputs: XBAR-transposing DMAs straight from DRAM ---
            wT = {}
            bias = {}
            for nm, wdram, bdram in (("v", Wv, bv), ("k", Wk, bk), ("q", Wq, bq)):
                bt = qkv.tile([128, 2], f32, name=f"b{nm}_{rep}")
                nc.sync.dma_start(bt, bdram[:].rearrange("(t p) -> p t", p=128))
                bias[nm] = bt
                wt = qkv.tile([128, 4, DG], bf16, name=f"wT{nm}_{rep}")
                nc.sync.dma_start_transpose(wt, wdram[:, :])
                wT[nm] = wt
            inT = {}
            for nm, dram in (("k", key), ("q", qry), ("v", val)):
                it = qkv.tile([128, 4, S], bf16, name=f"inT{nm}_{rep}")
                nc.sync.dma_start_transpose(it, dram[:, :])
                inT[nm] = it

            # --- projections: dstT2[p][c, s] = sum_cin W[c, cin] inT[cin, s] ---
            def proj(nm, dstT2):
                with tc.tile_pool(
                    name=f"ps_{nm}_{rep}", bufs=4, space="PSUM"
                ) as tps:
                    for p in (0, 1):
                        for jb in range(4):
                            ps = tps.tile([128, 512], f32, tag="pj")
                            for cc in range(4):
                                nc.tensor.matmul(
                                    ps,
                                    wT[nm][:, cc, p * 128 : (p + 1) * 128],
                                    inT[nm][:, cc, jb * 512 : (jb + 1) * 512],
                                    start=(cc == 0),
                                    stop=(cc == 3),
                                )
                            nc.vector.tensor_scalar_add(
                                dstT2[p][:, jb * 512 : (jb + 1) * 512],
                                ps,
                                bias[nm][:, p : p + 1],
                            )

            proj("k", kT2)
            proj("q", qT2)
            # natural-layout q (bf16) for the 1/Z row scaling: convert + XBAR
            with tc.tile_pool(name=f"qb_{rep}", bufs=2) as qbp:
                for p in (0, 1):
                    qb = qbp.tile([128, S], bf16, tag="qb")
                    nc.vector.tensor_copy(qb, qT2[p].bitcast(f32))
                    nc.sync.dma_start_transpose(qn[p], qb)
            proj("v", vT2)
            for p in (0, 1):
                nc.vector.tensor_add(fus[p], kT2[p].bitcast(f32), vT2[p].bitcast(f32))

            # --- attention: m-outer, sc ping-pong, GT pipelined one iter back ---
            for p in (0, 1):
                with (
                    tc.tile_pool(name=f"aps{p}_{rep}", bufs=1, space="PSUM") as aps,
                    tc.tile_pool(name=f"pp{p}_{rep}", bufs=6) as ppool,
                    tc.tile_pool(name=f"sm{p}_{rep}", bufs=3) as smp,
                ):
                    gt2 = aps.tile([128, S], f32, name=f"gt{p}_{rep}")
                    sc = [
                        aps.tile([128, 1024], f32, name=f"sc{p}{h}_{rep}")
                        for h in (0, 1)
                    ]
                    prev = None  # (wsd0, wsd1, pts0, pts1, first)

                    def emit_gt(pv, last):
                        w0, w1, pt0, pt1, first = pv
                        for jq in range(4):
                            for h, wh, pth in ((0, w0, pt0), (1, w1, pt1)):
                                nc.tensor.matmul(
                                    gt2[64 * h : 64 * (h + 1), jq * 512 : (jq + 1) * 512],
                                    wh,
                                    pth[:, jq * 512 : (jq + 1) * 512],
                                    start=first,
                                    stop=last,
                                    skip_group_check=True,
                                )

                    # probe phases: 12=score mms only; 13=+exp, no accum/Z/wsd;
                    # 14=+exp with accum, no wsd/GT; 2=no GT; >=3(or 4)=full
                    if phase >= 2:
                        for m, src in ((0, kT2[p]), (1, vT2[p])):
                            for ic in range(16):
                                pts = [
                                    ppool.tile(
                                        [128, S], bf16, tag=f"pt{h}",
                                        name=f"pt{h}_{m}_{ic}_{rep}",
                                    )
                                    for h in (0, 1)
                                ]
                                z2 = smp.tile(
                                    [128, 2], f32, tag="z2", name=f"z2_{m}_{ic}_{rep}"
                                )
                                lhs = [
                                    qT2[p][64 * h : 64 * (h + 1), ic * 128 : (ic + 1) * 128]
                                    for h in (0, 1)
                                ]

                                def mmpair(h, jh):
                                    for jq in (0, 1):
                                        nc.tensor.matmul(
                                            sc[h][:, jq * 512 : (jq + 1) * 512],
                                            lhs[h],
                                            src[
                                                64 * h : 64 * (h + 1),
                                                (2 * jh + jq) * 512 : (2 * jh + jq + 1) * 512,
                                            ],
                                            start=True,
                                            stop=True,
                                        )

                                def expi(h, jh):
                                    if phase == 12:
                                        return
                                    nc.scalar.activation(
                                        pts[h][:, jh * 1024 : (jh + 1) * 1024],
                                        sc[h],
                                        FT.Exp,
                                    )

                                def zred(h):
                                    # Z per head: one full-row DVE reduce (bf16 2x)
                                    nc.vector.tensor_reduce(
                                        z2[:, h : h + 1], pts[h], axis=AXL.X, op=ALU.add
                                    )

                                mmpair(0, 0)
                                mmpair(1, 0)
                                expi(0, 0)
                                mmpair(0, 1)
                                expi(1, 0)
                                mmpair(1, 1)
                                expi(0, 1)
                                if prev is not None and phase >= 3 and phase < 10:
                                    emit_gt(prev, last=False)
                                expi(1, 1)
                                if phase in (12, 13, 14):
                                    continue
                                zred(0)
                                zred(1)
                                rs2 = smp.tile(
                                    [128, 2], f32, tag="rs", name=f"rs_{m}_{ic}_{rep}"
                                )
                                nc.vector.reciprocal(rs2, z2)
                                wsd = []
                                for h in (0, 1):
                                    w = smp.tile(
                                        [128, HD], bf16, tag=f"w{h}",
                                        name=f"w{h}_{m}_{ic}_{rep}",
                                    )
                                    nc.vector.tensor_scalar_mul(
                                        w,
                                        qn[p][:, ic, 64 * h : 64 * (h + 1)],
                                        rs2[:, h : h + 1],
                                    )
                                    wsd.append(w)
                                prev = (
                                    wsd[0], wsd[1], pts[0], pts[1],
                                    (m == 0 and ic == 0),
                                )
                        if phase >= 3 and phase < 10:
                            emit_gt(prev, last=True)
                        prev = None

                    scr = smp.tile([128, S], f32, tag="scr")
                    nc.vector.tensor_mul(
                        scr, gt2 if (3 <= phase < 10) else fus[p], fus[p]
                    )
                    nc.vector.tensor_reduce(
                        outsb[:, p : p + 1], scr, axis=AXL.X, op=ALU.add
                    )
                nc.sync.dma_start(
                    out[:].rearrange("(t q) -> t q", t=2)[p].unsqueeze(1),
                    outsb[:, p : p + 1],
                )

    nc.compile()
    return nc


_CACHE = {}


def _program(phase=4, reps=1, timing=False):
    key = (phase, reps, timing)
    if key not in _CACHE:
        _CACHE[key] = build_program(phase=phase, reps=reps, timing=timing)
    return _CACHE[key]


def make_in_maps(inputs):
    import ml_dtypes

    b16 = ml_dtypes.bfloat16
    v = np.asarray(inputs["value"], dtype=np.float32).astype(b16)
    k = np.asarray(inputs["key"], dtype=np.float32).astype(b16)
    q = np.asarray(inputs["query"], dtype=np.float32).astype(b16)
    Wv = np.asarray(inputs["Wv"], dtype=np.float32)
    Wk = np.asarray(inputs["Wk"], dtype=np.float32)
    Wq = np.asarray(inputs["Wq"], dtype=np.float32)
    bv = np.asarray(inputs["bv"], dtype=np.float32)
    bk = np.asarray(inputs["bk"], dtype=np.float32)
    bq = np.asarray(inputs["bq"], dtype=np.float32)
    in_maps = []
    for c in range(8):
        b, g = divmod(c, 2)
        sl = slice(g * DG, (g + 1) * DG)
        in_maps.append(
            {
                "value": np.ascontiguousarray(v[b]),
                "key": np.ascontiguousarray(k[b]),
                "query": np.ascontiguousarray(q[b]),
                "Wv": np.ascontiguousarray(Wv[sl]).astype(b16),
                "bv": np.ascontiguousarray(bv[sl]),
                "Wk": np.ascontiguousarray(Wk[sl]).astype(b16),
                "bk": np.ascontiguousarray(bk[sl]),
                # softmax 1/sqrt(HD) folded into the query projection
                "Wq": (np.ascontiguousarray(Wq[sl]) * 0.125).astype(b16),
                "bq": np.ascontiguousarray(bq[sl]) * 0.125,
            }
        )
    return in_maps


def gather_out(results):
    out = np.zeros((B, H * HD), np.float32)
    for c in range(8):
        b, g = divmod(c, 2)
        # compensate the folded 1/8 query scale
        out[b, g * DG : (g + 1) * DG] = results[c]["out"] * 8.0
    return out


def run_sharded(inputs, trace=False, **kwargs):
    from concourse.bass_utils import run_bass_kernel_spmd

    nc = _program()
    res = run_bass_kernel_spmd(
        nc, make_in_maps(inputs), core_ids=list(range(8)), trace=trace, **kwargs
    )
    return gather_out(res.results), res


def kernel(**inputs):
    out, _ = run_sharded(inputs)
    return out
